# revision 44
# baseline (speedup 1.0000x reference)
"""Trainium2 Bass kernel for a causal single-head attention block.

Reference computation (per batch b):
    q = x @ Wq + bq ; k = x @ Wk + bk ; v = x @ Wv + bv      (x: [S, D])
    logits = q @ k.T  (causal masked), probs = softmax(logits / sqrt(128))
    out = concat([x, probs @ v], axis=-1)                     -> [S, D+128]

Shapes are hardcoded: B=4, S=2048, D=1024, feature size 128, 8 NeuronCores.

Sharding (SPMD, one compiled graph for all 8 cores):
  core c -> batch b = c//2, interleave parity h = c%2.
  Each core computes the 8 query blocks (128 rows each) at global block
  positions {2j + h : j in 0..7} of its batch, and the K/V projection over
  the full 2048-row sequence of that batch.  h=1 cores get a pair-swapped
  block order of x^T so the causal mask structure is identical across all
  cores (SPMD); the two 128x128 mask tiles are per-core input data.

Layout: OWN-PARITY-FIRST.  The core's own (query) blocks occupy SBUF/DRAM
columns [0:1024] in chunk order (128,128,256,512); the other parity's
blocks occupy [1024:2048].  Causal local block position ki maps to sbuf
block p(ki) = ki//2 + 8*(ki%2).  This makes the Q projection read early,
contiguous columns -- the logits/exp/PV pipeline starts while the second
half of x^T is still loading.  Input chunks are spread over the three DMA
trigger queues (sync / scalar / gpsimd) so their latencies overlap.

Bias handling (exact algebra, no device cost):
  - bk: q.bk is constant over keys -> cancels in softmax.  Dropped.
  - bv: probs rows sum to 1 -> read += bv on the host.
  - bq: kept, applied on the Q projection PSUM evacuation (ACT affine).

Precision scheme (error budget is ~2e-2; measured ~2.5e-3 full):
  - x^T is shipped and used as fp8e4 (halves the input DMA, 2 MB/core)
  - Wq/Wk/Wv are fp8e4 scaled by x16 on the host (avoids fp8 subnormals);
    projections run in DoubleRow perf mode (2 fp8 k-tiles per pass)
  - descaling is free: Q evac ACT scale 1/16; K's x16 folds into the exp
    scale; V's x16 folds into the ones column (16.0) of the PV matmul
  - logits / PV matmuls stay bf16

On-chip scheme:
  - kT, qT = W.T @ x^T  ([128 feat, rows]) directly in matmul layout
  - logits computed transposed: logitsT[k, q] = kT_blk.T @ qT  -> after the
    masked exp, expT[k, q] is directly the stationary operand of PV
  - v natural layout via 16 PE transposes of vT; read_aug[q, 0:129] =
    expT.T @ [v | 16] accumulates read + softmax denominators together
  - normalize with reciprocal * per-partition scale, write bf16
  - x passthrough half of the output is assembled on the HOST.
"""

import contextlib
import math

import numpy as np
import ml_dtypes

import concourse.bass as bass
import concourse.tile as tile
from concourse import bacc, mybir
from concourse.bass_utils import run_bass_kernel_spmd
from concourse.masks import make_identity

N_CORES = 8
B = 4
S = 2048  # sequence length per batch
D = 1024  # model dim
F = 128  # q/k/v feature size
NQT = 8  # local query subtiles of 128 rows
NKT = 16  # key tiles of 128 rows (full sequence)
QROWS = NQT * 128  # 1024 local query rows per core
SCALE = 1.0 / math.sqrt(F)
NEG = -1.0e9
WSCALE = 16.0  # host-side fp8 weight scale

FP32 = mybir.dt.float32
BF16 = mybir.dt.bfloat16
FP8 = mybir.dt.float8e4
BF16_NP = ml_dtypes.bfloat16
FP8_NP = mybir.dt.np(FP8)
DR = mybir.MatmulPerfMode.DoubleRow

_compiled = {}

# xT DMA chunking over SBUF columns (own-parity blocks first).  Finest
# first so the first projection matmuls start as early as possible.  Host
# lays xT out chunk-major so each chunk is one contiguous DRAM block.
CHUNKS = ((0, 64), (64, 192), (256, 256), (512, 512), (1024, 512), (1536, 512))
N_EVEN_CHUNKS = 4  # first 4 chunks cover the own (query) blocks [0:1024]


def p_of(ki):
    """Causal local block position -> sbuf block position."""
    return ki // 2 + 8 * (ki % 2)


def _build(niter=1, bias_q=False):
    nc = bacc.Bacc("TRN2", target_bir_lowering=False, debug=False, num_devices=N_CORES)

    xT_ext = nc.dram_tensor("xT", [D * S], FP8, kind="ExternalInput")
    wq_ext = nc.dram_tensor("wq", [128, 8, 128], FP8, kind="ExternalInput")
    wk_ext = nc.dram_tensor("wk", [128, 8, 128], FP8, kind="ExternalInput")
    wv_ext = nc.dram_tensor("wv", [128, 8, 128], FP8, kind="ExternalInput")
    bq_ext = (
        nc.dram_tensor("bq", [128, 1], FP32, kind="ExternalInput") if bias_q else None
    )
    mask_ext = nc.dram_tensor("masks", [128, 2, 128], BF16, kind="ExternalInput")
    out_ext = nc.dram_tensor("out", [QROWS, F], BF16, kind="ExternalOutput")

    with tile.TileContext(nc) as tc:
        with (
            tc.tile_pool(name="persist", bufs=1) as P,
            tc.tile_pool(name="ps_proj", bufs=2, space="PSUM") as ps_proj,
            tc.tile_pool(name="ps_log", bufs=2, space="PSUM") as ps_log,
            tc.tile_pool(name="ps_tp", bufs=1, space="PSUM") as ps_tp,
            tc.tile_pool(name="ps_read", bufs=1, space="PSUM") as ps_read,
            tc.For_i(0, niter) if niter > 1 else contextlib.nullcontext(),
        ):
            # ---- persistent SBUF tiles ----
            xT_sb = P.tile([128, 8, S], FP8)  # [d%128, d//128, sbuf col]
            wq_sb = P.tile([128, 8, 128], FP8)
            wk_sb = P.tile([128, 8, 128], FP8)
            wv_sb = P.tile([128, 8, 128], FP8)
            bq_sb = P.tile([128, 1], FP32) if bias_q else None
            mask_sb = P.tile([128, 2, 128], BF16)
            ident = P.tile([128, 128], BF16)
            zero_sb = P.tile([128, 1], FP32)
            kT_sb = P.tile([128, S], BF16)  # [feat, sbuf col] (x16 scaled)
            qT_sb = P.tile([128, QROWS], BF16)  # [feat, local q] (clean)
            vT_sb = P.tile([128, S], BF16)  # [feat, sbuf col] (x16 scaled)
            vaug_sb = P.tile([128, NKT, 144], FP8)  # [s%128, sbuf blk, v|16]
            expT_sb = P.tile([128, NKT, QROWS], FP8)  # [s%128, sbuf blk, q]
            read_sb = P.tile([128, NQT, 128], BF16)
            recip_sb = P.tile([128, NQT, 1], FP32)

            # ---- input DMAs spread over the three trigger queues ----
            xt_srcs = []
            base = 0
            for off, w in CHUNKS:
                n = 128 * 8 * w
                xt_srcs.append(
                    xT_ext[base:base + n].rearrange("(p t w) -> p t w", p=128, t=8)
                )
                base += n

            def xdma(eng, i):
                off, w = CHUNKS[i]
                eng.dma_start(xT_sb[:, :, off:off + w], xt_srcs[i])

            # HWDGE triggers serialize (~630 ns each): wk first (gates the
            # first matmul), then the x^T chunks in consumption order,
            # alternating sync/scalar rings.
            nc.sync.dma_start(wk_sb[:], wk_ext[:])
            xdma(nc.scalar, 0)
            xdma(nc.sync, 1)
            xdma(nc.scalar, 2)
            xdma(nc.sync, 3)
            xdma(nc.scalar, 4)
            xdma(nc.sync, 5)
            # SWDGE (gpsimd) runs on a parallel descriptor path: the
            # later-needed weights and masks go there.
            nc.gpsimd.dma_start(wq_sb[:], wq_ext[:])
            nc.gpsimd.dma_start(mask_sb[:], mask_ext[:])
            nc.gpsimd.dma_start(wv_sb[:], wv_ext[:])
            if bias_q:
                nc.gpsimd.dma_start(bq_sb[:], bq_ext[:])
            make_identity(nc, ident[:])
            nc.vector.memset(zero_sb[:], -2.0)
            nc.vector.memset(vaug_sb[:, :, 128:129], WSCALE)

            # ---- projections (fp8 DoubleRow: 2 contraction k-tiles per
            # pass, 4 passes for D=1024), PSUM evac on DVE except Q (ACT
            # applies bias + 1/16 descale) ----
            def proj(w_sb, sl, dst_ap, evac):
                pp = ps_proj.tile([128, sl.stop - sl.start], FP32, tag="proj")
                for u in range(4):
                    nc.tensor.matmul(
                        pp[:],
                        w_sb[:, 2 * u:2 * u + 2, :],
                        xT_sb[:, 2 * u:2 * u + 2, sl],
                        start=(u == 0),
                        stop=(u == 3),
                        perf_mode=DR,
                    )
                evac(dst_ap, pp)

            def evac_dve(dst, pp):
                nc.vector.tensor_copy(dst, pp[:])

            def evac_act(dst, pp):
                nc.scalar.copy(dst, pp[:])

            if bias_q:
                def evac_q(dst, pp):
                    nc.scalar.activation(
                        dst, pp[:], mybir.ActivationFunctionType.Identity,
                        bias=bq_sb[:], scale=1.0 / WSCALE,
                    )
            else:
                def evac_q(dst, pp):
                    nc.vector.tensor_scalar_mul(dst, pp[:], 1.0 / WSCALE)

            # own half, in chunk order: K then Q per chunk (Q unblocks the
            # whole logits pipeline), then V
            for i in range(N_EVEN_CHUNKS):
                off, w = CHUNKS[i]
                sl = slice(off, off + w)
                proj(wk_sb, sl, kT_sb[:, sl], evac_dve)
                proj(wq_sb, sl, qT_sb[:, sl], evac_q)
            for off, w in ((0, 512), (512, 512)):
                sl = slice(off, off + w)
                proj(wv_sb, sl, vT_sb[:, sl], evac_act)
                for pb in range(off // 128, (off + w) // 128):
                    pt = ps_tp.tile([128, 128], BF16, tag="tp")
                    nc.tensor.transpose(
                        pt[:], vT_sb[:, pb * 128:(pb + 1) * 128], ident[:]
                    )
                    nc.vector.tensor_copy(vaug_sb[:, pb, 0:128], pt[:])
            # other half: K then V per 512-chunk
            for off in (1024, 1536):
                sl = slice(off, off + 512)
                proj(wk_sb, sl, kT_sb[:, sl], evac_dve)
                proj(wv_sb, sl, vT_sb[:, sl], evac_act)
                for pb in range(off // 128, (off + 512) // 128):
                    pt = ps_tp.tile([128, 128], BF16, tag="tp")
                    nc.tensor.transpose(
                        pt[:], vT_sb[:, pb * 128:(pb + 1) * 128], ident[:]
                    )
                    nc.vector.tensor_copy(vaug_sb[:, pb, 0:128], pt[:])

            # ---- logits^T, exp, 0/1-mask, with each query block's PV
            # group emitted right after its last dependency (ki = 2j+1) so
            # PV matmuls don't queue behind later logits on the in-order
            # PE engine.  Masking expT on DVE keeps the mask OFF the
            # logits->exp critical chain (unmasked exp is safe: post-scale
            # logits are O(5)).  masks hold 0/1 here. ----
            out_read = out_ext[:].rearrange("(g p) c -> p g c", p=128)
            out_dmas = []
            prr = ps_read.tile([128, 2, 132], FP32, tag="read")
            expT_x = expT_sb[:].rearrange("p (two g) q -> p two g q", two=2)
            vaug_x = vaug_sb[:].rearrange("p (two g) c -> p two g c", two=2)
            for ki in range(NKT):
                p = p_of(ki)
                qs = 128 * (ki // 2)
                kb = slice(p * 128, (p + 1) * 128)
                off = qs
                while off < QROWS:
                    w = min(1024, QROWS - off)
                    pl = ps_log.tile([128, 1024], FP32, tag="log")
                    for so in range(0, w, 512):
                        sw = min(512, w - so)
                        nc.tensor.matmul(
                            pl[:, so:so + sw],
                            kT_sb[:, kb], qT_sb[:, off + so:off + so + sw],
                            start=True, stop=True,
                        )
                    nc.scalar.activation(
                        expT_sb[:, p, off:off + w], pl[:, 0:w],
                        mybir.ActivationFunctionType.Exp, bias=zero_sb[:],
                        scale=SCALE / WSCALE,
                    )
                    off += w
                nc.vector.tensor_mul(
                    expT_sb[:, p, qs:qs + 128],
                    expT_sb[:, p, qs:qs + 128],
                    mask_sb[:, ki % 2, :],
                )

                if ki % 2 == 0:
                    continue
                # ---- PV + denominators + normalize for query block j ----
                j = ki // 2
                pr = prr[:, j % 2, 0:129]
                # sbuf blocks [0..j] and [8..8+j]; DoubleRow-pair block m
                # with block m+8 (fp8: 2 key tiles per pass, no leftovers)
                jb = slice(j * 128, (j + 1) * 128)
                for m in range(j + 1):
                    nc.tensor.matmul(
                        pr,
                        expT_x[:, :, m, jb],
                        vaug_x[:, :, m, 0:129],
                        start=(m == 0),
                        stop=(m == j),
                        perf_mode=DR,
                    )
                nc.vector.reciprocal(recip_sb[:, j, :], pr[:, 128:129])
                nc.vector.tensor_scalar_mul(
                    read_sb[:, j, :], pr[:, 0:128], recip_sb[:, j, :]
                )
                # batched output writes: [0:4] on the parallel SWDGE path
                # mid-kernel, then [4:7] and the critical [7] on HWDGE
                if j == 3:
                    out_dmas.append(
                        nc.gpsimd.dma_start(
                            out=out_read[:, 0:4, :], in_=read_sb[:, 0:4, :]
                        )
                    )
                elif j == 6:
                    out_dmas.append(
                        nc.sync.dma_start(
                            out=out_read[:, 4:7, :], in_=read_sb[:, 4:7, :]
                        )
                    )
                elif j == 7:
                    out_dmas.append(
                        nc.sync.dma_start(
                            out=out_read[:, 7, :], in_=read_sb[:, 7, :]
                        )
                    )

            if niter > 1:
                # timing mode: fence the iteration on every output DMA so the
                # per-iteration span matches a single-shot NEFF exec span
                fence = nc.vector.memset(zero_sb[:], -2.0)
                for d in out_dmas:
                    tile.add_dep_helper(fence.ins, d.ins, sync=True, reason="fence")

    nc.compile()
    return nc


def _get_compiled(niter=1, bias_q=False):
    key = f"nc{niter}b{int(bias_q)}"
    if key not in _compiled:
        _compiled[key] = _build(niter, bias_q=bias_q)
    return _compiled[key]


def _make_in_maps(inputs, Wq, bq, Wk, bk, Wv, bv):
    x = np.asarray(inputs, dtype=np.float32)
    assert x.shape == (B, S, D)

    def prep_w(w):
        w = (np.asarray(w, dtype=np.float32) * WSCALE).astype(FP8_NP)
        return np.ascontiguousarray(w.reshape(8, 128, 128).transpose(1, 0, 2))

    wq_np, wk_np, wv_np = prep_w(Wq), prep_w(Wk), prep_w(Wv)
    bias_q = bool(np.any(np.asarray(bq)))
    bq_np = np.asarray(bq, np.float32).reshape(128, 1)

    # 0/1 masks[k, slot, q]: slot 0 = diagonal block (triangular), slot 1 =
    # the extra block (all-0 for h=0, all-1 for h=1); multiplied into expT
    kk = np.arange(128)[:, None]
    qq = np.arange(128)[None, :]
    tri = (qq >= kk).astype(np.float32)
    m_h = []
    for h in range(2):
        other = np.full((128, 128), float(h), np.float32)
        m = np.stack([tri, other], axis=1)  # [k, slot, q]
        m_h.append(np.ascontiguousarray(m.astype(BF16_NP)))

    in_maps = []
    for c in range(N_CORES):
        b, h = divmod(c, 2)
        xb = x[b]  # [S, D]
        # sbuf block order: own parity blocks (2j+h) first, then the rest
        order = np.concatenate([np.arange(h, NKT, 2), np.arange(1 - h, NKT, 2)])
        xb_local = xb.reshape(NKT, 128, D)[order].reshape(S, D)
        xT_full = xb_local.T.astype(FP8_NP)  # [D, S] = [(t p), s]
        xT_tps = xT_full.reshape(8, 128, S).transpose(1, 0, 2)  # [p, t, s]
        xT = np.concatenate(
            [xT_tps[:, :, off:off + w].reshape(-1) for off, w in CHUNKS]
        )  # chunk-major flat, each chunk contiguous [p, t, w]
        m = {
            "xT": xT,
            "wq": wq_np,
            "wk": wk_np,
            "wv": wv_np,
            "masks": m_h[h],
        }
        if bias_q:
            m["bq"] = bq_np
        in_maps.append(m)
    return in_maps, bias_q


def _gather(results, x, bv):
    """Assemble full output: exact host passthrough + device read part."""
    out = np.empty((B, S, D + F), dtype=np.float32)
    out[:, :, :D] = x
    bv = np.asarray(bv, np.float32)
    for c in range(N_CORES):
        b, h = divmod(c, 2)
        oc = np.asarray(results[c]["out"], dtype=np.float32).reshape(NQT, 128, F)
        for j in range(NQT):
            g = 2 * j + h
            out[b, g * 128:(g + 1) * 128, D:] = oc[j] + bv
    return out


def run(inputs, Wq, bq, Wk, bk, Wv, bv, trace=False):
    """Build (cached), run on 8 cores, gather. Returns (output, results)."""
    in_maps, bias_q = _make_in_maps(inputs, Wq, bq, Wk, bk, Wv, bv)
    nc = _get_compiled(bias_q=bias_q)
    x = np.asarray(inputs, dtype=np.float32)
    if trace:
        try:
            res = run_bass_kernel_spmd(nc, in_maps, list(range(N_CORES)), trace=True)
            return _gather(res.results, x, bv), res
        except Exception as e:  # profiling hook unavailable etc.
            print(f"trace run failed ({e!r}); falling back to untraced run")
    res = run_bass_kernel_spmd(nc, in_maps, list(range(N_CORES)))
    return _gather(res.results, x, bv), res


def kernel(inputs, Wq, bq, Wk, bk, Wv, bv):
    out, _ = run(inputs, Wq, bq, Wk, bk, Wv, bv, trace=False)
    return out


# revision 45
# speedup vs baseline: 1.7271x; 1.7271x over previous
"""Trainium2 Bass kernel for a causal single-head attention block.

Reference computation (per batch b):
    q = x @ Wq + bq ; k = x @ Wk + bk ; v = x @ Wv + bv      (x: [S, D])
    logits = q @ k.T  (causal masked), probs = softmax(logits / sqrt(128))
    out = concat([x, probs @ v], axis=-1)                     -> [S, D+128]

Shapes are hardcoded: B=4, S=2048, D=1024, feature size 128, 8 NeuronCores.

Sharding (SPMD, one compiled graph for all 8 cores):
  core c -> batch b = c//2, interleave parity h = c%2.
  Each core computes the 8 query blocks (128 rows each) at global block
  positions {2j + h : j in 0..7} of its batch, and the K/V projection over
  the full 2048-row sequence of that batch.  h=1 cores get a pair-swapped
  block order of x^T so the causal mask structure is identical across all
  cores (SPMD); the two 128x128 mask tiles are per-core input data.

Layout: OWN-PARITY-FIRST.  The core's own (query) blocks occupy SBUF/DRAM
columns [0:1024] in chunk order (128,128,256,512); the other parity's
blocks occupy [1024:2048].  Causal local block position ki maps to sbuf
block p(ki) = ki//2 + 8*(ki%2).  This makes the Q projection read early,
contiguous columns -- the logits/exp/PV pipeline starts while the second
half of x^T is still loading.  Input chunks are spread over the three DMA
trigger queues (sync / scalar / gpsimd) so their latencies overlap.

Bias handling (exact algebra, no device cost):
  - bk: q.bk is constant over keys -> cancels in softmax.  Dropped.
  - bv: probs rows sum to 1 -> read += bv on the host.
  - bq: kept, applied on the Q projection PSUM evacuation (ACT affine).

Precision scheme (error budget is ~2e-2; measured ~2.5e-3 full):
  - x^T is shipped and used as fp8e4 (halves the input DMA, 2 MB/core)
  - Wq/Wk/Wv are fp8e4 scaled by x16 on the host (avoids fp8 subnormals);
    projections run in DoubleRow perf mode (2 fp8 k-tiles per pass)
  - descaling is free: Q evac ACT scale 1/16; K's x16 folds into the exp
    scale; V's x16 folds into the ones column (16.0) of the PV matmul
  - logits / PV matmuls stay bf16

On-chip scheme:
  - kT, qT = W.T @ x^T  ([128 feat, rows]) directly in matmul layout
  - logits computed transposed: logitsT[k, q] = kT_blk.T @ qT  -> after the
    masked exp, expT[k, q] is directly the stationary operand of PV
  - v natural layout via 16 PE transposes of vT; read_aug[q, 0:129] =
    expT.T @ [v | 16] accumulates read + softmax denominators together
  - normalize with reciprocal * per-partition scale, write bf16
  - x passthrough half of the output is assembled on the HOST.
"""

import contextlib
import math

import numpy as np
import ml_dtypes

import concourse.bass as bass
import concourse.tile as tile
from concourse import bacc, mybir
from concourse.bass_utils import run_bass_kernel_spmd
from concourse.masks import make_identity

N_CORES = 8
B = 4
S = 2048  # sequence length per batch
D = 1024  # model dim
F = 128  # q/k/v feature size
NQT = 8  # local query subtiles of 128 rows
NKT = 16  # key tiles of 128 rows (full sequence)
QROWS = NQT * 128  # 1024 local query rows per core
SCALE = 1.0 / math.sqrt(F)
NEG = -1.0e9
WSCALE = 16.0  # host-side fp8 weight scale

FP32 = mybir.dt.float32
BF16 = mybir.dt.bfloat16
FP8 = mybir.dt.float8e4
BF16_NP = ml_dtypes.bfloat16
FP8_NP = mybir.dt.np(FP8)
DR = mybir.MatmulPerfMode.DoubleRow

_compiled = {}

# xT DMA chunking over SBUF columns (own-parity blocks first).  Finest
# first so the first projection matmuls start as early as possible.  Host
# lays xT out chunk-major so each chunk is one contiguous DRAM block.
CHUNKS = ((0, 64), (64, 192), (256, 256), (512, 512), (1024, 512), (1536, 512))
N_EVEN_CHUNKS = 4  # first 4 chunks cover the own (query) blocks [0:1024]


def p_of(ki):
    """Causal local block position -> sbuf block position."""
    return ki // 2 + 8 * (ki % 2)


def _build(niter=1, bias_q=False):
    nc = bacc.Bacc("TRN2", target_bir_lowering=False, debug=False, num_devices=N_CORES)

    xT_ext = nc.dram_tensor("xT", [D * S], FP8, kind="ExternalInput")
    wq_ext = nc.dram_tensor("wq", [128, 8, 128], FP8, kind="ExternalInput")
    wk_ext = nc.dram_tensor("wk", [128, 8, 128], FP8, kind="ExternalInput")
    wv_ext = nc.dram_tensor("wv", [128, 8, 128], FP8, kind="ExternalInput")
    bq_ext = (
        nc.dram_tensor("bq", [128, 1], FP32, kind="ExternalInput") if bias_q else None
    )
    mask_ext = nc.dram_tensor("masks", [128, 2, 128], BF16, kind="ExternalInput")
    out_ext = nc.dram_tensor("out", [QROWS, F], BF16, kind="ExternalOutput")

    with tile.TileContext(nc) as tc:
        with (
            tc.tile_pool(name="persist", bufs=1) as P,
            tc.tile_pool(name="ps_proj", bufs=2, space="PSUM") as ps_proj,
            tc.tile_pool(name="ps_log", bufs=2, space="PSUM") as ps_log,
            tc.tile_pool(name="ps_tp", bufs=1, space="PSUM") as ps_tp,
            tc.tile_pool(name="ps_read", bufs=1, space="PSUM") as ps_read,
            tc.For_i(0, niter) if niter > 1 else contextlib.nullcontext(),
        ):
            # ---- persistent SBUF tiles ----
            xT_sb = P.tile([128, 8, S], FP8)  # [d%128, d//128, sbuf col]
            wq_sb = P.tile([128, 8, 128], FP8)
            wk_sb = P.tile([128, 8, 128], FP8)
            wv_sb = P.tile([128, 8, 128], FP8)
            bq_sb = P.tile([128, 1], FP32) if bias_q else None
            mask_sb = P.tile([128, 2, 128], BF16)
            ident = P.tile([128, 128], BF16)
            zero_sb = P.tile([128, 1], FP32)
            kT_sb = P.tile([128, S], BF16)  # [feat, sbuf col] (x16 scaled)
            qT_sb = P.tile([128, QROWS], BF16)  # [feat, local q] (clean)
            vT_sb = P.tile([128, S], BF16)  # [feat, sbuf col] (x16 scaled)
            vaug_sb = P.tile([128, NKT, 144], FP8)  # [s%128, sbuf blk, v|16]
            expT_sb = P.tile([128, NKT, QROWS], FP8)  # [s%128, sbuf blk, q]
            read_sb = P.tile([128, NQT, 128], BF16)
            recip_sb = P.tile([128, NQT, 1], FP32)

            # ---- input DMAs spread over the three trigger queues ----
            xt_srcs = []
            base = 0
            for off, w in CHUNKS:
                n = 128 * 8 * w
                xt_srcs.append(
                    xT_ext[base:base + n].rearrange("(p t w) -> p t w", p=128, t=8)
                )
                base += n

            def xdma(eng, i):
                off, w = CHUNKS[i]
                eng.dma_start(xT_sb[:, :, off:off + w], xt_srcs[i])

            # HWDGE triggers serialize (~630 ns each): wk first (gates the
            # first matmul), then the x^T chunks in consumption order,
            # alternating sync/scalar rings.
            nc.sync.dma_start(wk_sb[:], wk_ext[:])
            xdma(nc.scalar, 0)
            xdma(nc.sync, 1)
            xdma(nc.scalar, 2)
            xdma(nc.sync, 3)
            xdma(nc.scalar, 4)
            xdma(nc.sync, 5)
            # SWDGE (gpsimd) runs on a parallel descriptor path: the
            # later-needed weights and masks go there.
            nc.gpsimd.dma_start(wq_sb[:], wq_ext[:])
            nc.gpsimd.dma_start(mask_sb[:], mask_ext[:])
            nc.gpsimd.dma_start(wv_sb[:], wv_ext[:])
            if bias_q:
                nc.gpsimd.dma_start(bq_sb[:], bq_ext[:])
            make_identity(nc, ident[:])
            nc.vector.memset(zero_sb[:], -2.0)
            nc.vector.memset(vaug_sb[:, :, 128:129], WSCALE)

            # ---- projections (fp8 DoubleRow: 2 contraction k-tiles per
            # pass, 4 passes for D=1024), PSUM evac on DVE except Q (ACT
            # applies bias + 1/16 descale) ----
            def proj(w_sb, sl, dst_ap, evac):
                pp = ps_proj.tile([128, sl.stop - sl.start], FP32, tag="proj")
                for u in range(4):
                    nc.tensor.matmul(
                        pp[:],
                        w_sb[:, 2 * u:2 * u + 2, :],
                        xT_sb[:, 2 * u:2 * u + 2, sl],
                        start=(u == 0),
                        stop=(u == 3),
                        perf_mode=DR,
                    )
                evac(dst_ap, pp)

            def evac_dve(dst, pp):
                nc.vector.tensor_copy(dst, pp[:])

            def evac_act(dst, pp):
                nc.scalar.copy(dst, pp[:])

            if bias_q:
                def evac_q(dst, pp):
                    nc.scalar.activation(
                        dst, pp[:], mybir.ActivationFunctionType.Identity,
                        bias=bq_sb[:], scale=1.0 / WSCALE,
                    )
            else:
                def evac_q(dst, pp):
                    nc.vector.tensor_scalar_mul(dst, pp[:], 1.0 / WSCALE)

            # own half, in chunk order: K then Q per chunk (Q unblocks the
            # whole logits pipeline), then V
            for i in range(N_EVEN_CHUNKS):
                off, w = CHUNKS[i]
                sl = slice(off, off + w)
                proj(wk_sb, sl, kT_sb[:, sl], evac_dve)
                proj(wq_sb, sl, qT_sb[:, sl], evac_q)
            for off, w in ((0, 512), (512, 512)):
                sl = slice(off, off + w)
                proj(wv_sb, sl, vT_sb[:, sl], evac_act)
                for pb in range(off // 128, (off + w) // 128):
                    pt = ps_tp.tile([128, 128], BF16, tag="tp")
                    nc.tensor.transpose(
                        pt[:], vT_sb[:, pb * 128:(pb + 1) * 128], ident[:]
                    )
                    nc.vector.tensor_copy(vaug_sb[:, pb, 0:128], pt[:])
            # other half: K then V per 512-chunk
            for off in (1024, 1536):
                sl = slice(off, off + 512)
                proj(wk_sb, sl, kT_sb[:, sl], evac_dve)
                proj(wv_sb, sl, vT_sb[:, sl], evac_act)
                for pb in range(off // 128, (off + 512) // 128):
                    pt = ps_tp.tile([128, 128], BF16, tag="tp")
                    nc.tensor.transpose(
                        pt[:], vT_sb[:, pb * 128:(pb + 1) * 128], ident[:]
                    )
                    nc.vector.tensor_copy(vaug_sb[:, pb, 0:128], pt[:])

            # ---- logits^T, exp, 0/1-mask, with each query block's PV
            # group emitted right after its last dependency (ki = 2j+1) so
            # PV matmuls don't queue behind later logits on the in-order
            # PE engine.  Masking expT on DVE keeps the mask OFF the
            # logits->exp critical chain (unmasked exp is safe: post-scale
            # logits are O(5)).  masks hold 0/1 here. ----
            out_read = out_ext[:].rearrange("(g p) c -> p g c", p=128)
            out_dmas = []
            expT_x = expT_sb[:].rearrange("p (two g) q -> p two g q", two=2)
            vaug_x = vaug_sb[:].rearrange("p (two g) c -> p two g c", two=2)
            for ki in range(NKT):
                p = p_of(ki)
                qs = 128 * (ki // 2)
                kb = slice(p * 128, (p + 1) * 128)
                off = qs
                while off < QROWS:
                    w = min(1024, QROWS - off)
                    pl = ps_log.tile([128, 1024], FP32, tag="log")
                    for so in range(0, w, 512):
                        sw = min(512, w - so)
                        nc.tensor.matmul(
                            pl[:, so:so + sw],
                            kT_sb[:, kb], qT_sb[:, off + so:off + so + sw],
                            start=True, stop=True,
                        )
                    nc.scalar.activation(
                        expT_sb[:, p, off:off + w], pl[:, 0:w],
                        mybir.ActivationFunctionType.Exp, bias=zero_sb[:],
                        scale=SCALE / WSCALE,
                    )
                    off += w
                nc.vector.tensor_mul(
                    expT_sb[:, p, qs:qs + 128],
                    expT_sb[:, p, qs:qs + 128],
                    mask_sb[:, ki % 2, :],
                )

                if ki % 2 == 0:
                    continue
            for ki in range(1, NKT, 2):
                # ---- PV + denominators + normalize for query block j ----
                j = ki // 2
                prt = ps_read.tile([128, 129], FP32, tag="read")
                pr = prt[:]
                # sbuf blocks [0..j] and [8..8+j]; DoubleRow-pair block m
                # with block m+8 (fp8: 2 key tiles per pass, no leftovers)
                jb = slice(j * 128, (j + 1) * 128)
                for m in range(j + 1):
                    nc.tensor.matmul(
                        pr,
                        expT_x[:, :, m, jb],
                        vaug_x[:, :, m, 0:129],
                        start=(m == 0),
                        stop=(m == j),
                        perf_mode=DR,
                    )
                nc.vector.reciprocal(recip_sb[:, j, :], pr[:, 128:129])
                nc.vector.tensor_scalar_mul(
                    read_sb[:, j, :], pr[:, 0:128], recip_sb[:, j, :]
                )
                # batched output writes: [0:4] on the parallel SWDGE path
                # mid-kernel, then [4:7] and the critical [7] on HWDGE
                if j == 3:
                    out_dmas.append(
                        nc.gpsimd.dma_start(
                            out=out_read[:, 0:4, :], in_=read_sb[:, 0:4, :]
                        )
                    )
                elif j == 7:
                    out_dmas.append(
                        nc.sync.dma_start(
                            out=out_read[:, 4:8, :], in_=read_sb[:, 4:8, :]
                        )
                    )

            if niter > 1:
                # timing mode: fence the iteration on every output DMA so the
                # per-iteration span matches a single-shot NEFF exec span
                fence = nc.vector.memset(zero_sb[:], -2.0)
                for d in out_dmas:
                    tile.add_dep_helper(fence.ins, d.ins, sync=True, reason="fence")

    nc.compile()
    return nc


def _get_compiled(niter=1, bias_q=False):
    key = f"nc{niter}b{int(bias_q)}"
    if key not in _compiled:
        _compiled[key] = _build(niter, bias_q=bias_q)
    return _compiled[key]


def _make_in_maps(inputs, Wq, bq, Wk, bk, Wv, bv):
    x = np.asarray(inputs, dtype=np.float32)
    assert x.shape == (B, S, D)

    def prep_w(w):
        w = (np.asarray(w, dtype=np.float32) * WSCALE).astype(FP8_NP)
        return np.ascontiguousarray(w.reshape(8, 128, 128).transpose(1, 0, 2))

    wq_np, wk_np, wv_np = prep_w(Wq), prep_w(Wk), prep_w(Wv)
    bias_q = bool(np.any(np.asarray(bq)))
    bq_np = np.asarray(bq, np.float32).reshape(128, 1)

    # 0/1 masks[k, slot, q]: slot 0 = diagonal block (triangular), slot 1 =
    # the extra block (all-0 for h=0, all-1 for h=1); multiplied into expT
    kk = np.arange(128)[:, None]
    qq = np.arange(128)[None, :]
    tri = (qq >= kk).astype(np.float32)
    m_h = []
    for h in range(2):
        other = np.full((128, 128), float(h), np.float32)
        m = np.stack([tri, other], axis=1)  # [k, slot, q]
        m_h.append(np.ascontiguousarray(m.astype(BF16_NP)))

    in_maps = []
    for c in range(N_CORES):
        b, h = divmod(c, 2)
        xb = x[b]  # [S, D]
        # sbuf block order: own parity blocks (2j+h) first, then the rest
        order = np.concatenate([np.arange(h, NKT, 2), np.arange(1 - h, NKT, 2)])
        xb_local = xb.reshape(NKT, 128, D)[order].reshape(S, D)
        xT_full = xb_local.T.astype(FP8_NP)  # [D, S] = [(t p), s]
        xT_tps = xT_full.reshape(8, 128, S).transpose(1, 0, 2)  # [p, t, s]
        xT = np.concatenate(
            [xT_tps[:, :, off:off + w].reshape(-1) for off, w in CHUNKS]
        )  # chunk-major flat, each chunk contiguous [p, t, w]
        m = {
            "xT": xT,
            "wq": wq_np,
            "wk": wk_np,
            "wv": wv_np,
            "masks": m_h[h],
        }
        if bias_q:
            m["bq"] = bq_np
        in_maps.append(m)
    return in_maps, bias_q


def _gather(results, x, bv):
    """Assemble full output: exact host passthrough + device read part."""
    out = np.empty((B, S, D + F), dtype=np.float32)
    out[:, :, :D] = x
    bv = np.asarray(bv, np.float32)
    for c in range(N_CORES):
        b, h = divmod(c, 2)
        oc = np.asarray(results[c]["out"], dtype=np.float32).reshape(NQT, 128, F)
        for j in range(NQT):
            g = 2 * j + h
            out[b, g * 128:(g + 1) * 128, D:] = oc[j] + bv
    return out


def run(inputs, Wq, bq, Wk, bk, Wv, bv, trace=False):
    """Build (cached), run on 8 cores, gather. Returns (output, results)."""
    in_maps, bias_q = _make_in_maps(inputs, Wq, bq, Wk, bk, Wv, bv)
    nc = _get_compiled(bias_q=bias_q)
    x = np.asarray(inputs, dtype=np.float32)
    if trace:
        try:
            res = run_bass_kernel_spmd(nc, in_maps, list(range(N_CORES)), trace=True)
            return _gather(res.results, x, bv), res
        except Exception as e:  # profiling hook unavailable etc.
            print(f"trace run failed ({e!r}); falling back to untraced run")
    res = run_bass_kernel_spmd(nc, in_maps, list(range(N_CORES)))
    return _gather(res.results, x, bv), res


def kernel(inputs, Wq, bq, Wk, bk, Wv, bv):
    out, _ = run(inputs, Wq, bq, Wk, bk, Wv, bv, trace=False)
    return out


# revision 48
# speedup vs baseline: 2.6628x; 1.5417x over previous
"""Trainium2 Bass kernel for a causal single-head attention block.

Reference computation (per batch b):
    q = x @ Wq + bq ; k = x @ Wk + bk ; v = x @ Wv + bv      (x: [S, D])
    logits = q @ k.T  (causal masked), probs = softmax(logits / sqrt(128))
    out = concat([x, probs @ v], axis=-1)                     -> [S, D+128]

Shapes are hardcoded: B=4, S=2048, D=1024, feature size 128, 8 NeuronCores.

Sharding (SPMD, one compiled graph for all 8 cores):
  core c -> batch b = c//2, interleave parity h = c%2.
  Each core computes the 8 query blocks (128 rows each) at global block
  positions {2j + h : j in 0..7} of its batch, and the K/V projection over
  the full 2048-row sequence of that batch.  h=1 cores get a pair-swapped
  block order of x^T so the causal mask structure is identical across all
  cores (SPMD); the two 128x128 mask tiles are per-core input data.

Layout: OWN-PARITY-FIRST.  The core's own (query) blocks occupy SBUF/DRAM
columns [0:1024] in chunk order (128,128,256,512); the other parity's
blocks occupy [1024:2048].  Causal local block position ki maps to sbuf
block p(ki) = ki//2 + 8*(ki%2).  This makes the Q projection read early,
contiguous columns -- the logits/exp/PV pipeline starts while the second
half of x^T is still loading.  Input chunks are spread over the three DMA
trigger queues (sync / scalar / gpsimd) so their latencies overlap.

Bias handling (exact algebra, no device cost):
  - bk: q.bk is constant over keys -> cancels in softmax.  Dropped.
  - bv: probs rows sum to 1 -> read += bv on the host.
  - bq: kept, applied on the Q projection PSUM evacuation (ACT affine).

Precision scheme (error budget is ~2e-2; measured ~2.5e-3 full):
  - x^T is shipped and used as fp8e4 (halves the input DMA, 2 MB/core)
  - Wq/Wk/Wv are fp8e4 scaled by x16 on the host (avoids fp8 subnormals);
    projections run in DoubleRow perf mode (2 fp8 k-tiles per pass)
  - descaling is free: Q evac ACT scale 1/16; K's x16 folds into the exp
    scale; V's x16 folds into the ones column (16.0) of the PV matmul
  - logits / PV matmuls stay bf16

On-chip scheme:
  - kT, qT = W.T @ x^T  ([128 feat, rows]) directly in matmul layout
  - logits computed transposed: logitsT[k, q] = kT_blk.T @ qT  -> after the
    masked exp, expT[k, q] is directly the stationary operand of PV
  - v natural layout via 16 PE transposes of vT; read_aug[q, 0:129] =
    expT.T @ [v | 16] accumulates read + softmax denominators together
  - normalize with reciprocal * per-partition scale, write bf16
  - x passthrough half of the output is assembled on the HOST.
"""

import contextlib
import math

import numpy as np
import ml_dtypes

import concourse.bass as bass
import concourse.tile as tile
from concourse import bacc, mybir
from concourse.bass_utils import run_bass_kernel_spmd
from concourse.masks import make_identity

N_CORES = 8
B = 4
S = 2048  # sequence length per batch
D = 1024  # model dim
F = 128  # q/k/v feature size
NQT = 8  # local query subtiles of 128 rows
NKT = 16  # key tiles of 128 rows (full sequence)
QROWS = NQT * 128  # 1024 local query rows per core
SCALE = 1.0 / math.sqrt(F)
NEG = -1.0e9
WSCALE = 16.0  # host-side fp8 weight scale

FP32 = mybir.dt.float32
BF16 = mybir.dt.bfloat16
FP8 = mybir.dt.float8e4
BF16_NP = ml_dtypes.bfloat16
FP8_NP = mybir.dt.np(FP8)
DR = mybir.MatmulPerfMode.DoubleRow

_compiled = {}

# xT DMA chunking over SBUF columns (own-parity blocks first).  Finest
# first so the first projection matmuls start as early as possible.  Host
# lays xT out chunk-major so each chunk is one contiguous DRAM block.
CHUNKS = ((0, 64), (64, 192), (256, 256), (512, 512), (1024, 512), (1536, 512))
N_EVEN_CHUNKS = 4  # first 4 chunks cover the own (query) blocks [0:1024]
# Wk is packed in front of the x^T stream (same [p, t, c] layout): the
# first DMA delivers the first matmul's stationary AND moving operands in
# one trigger.  Mega-column c holds wk for c<128, x column c-128 after.
XO = 128  # mega-column offset of x^T data
DCHUNKS = tuple((o + XO if i else 0, w + (XO if not i else 0))
                for i, (o, w) in enumerate(CHUNKS))


def p_of(ki):
    """Causal local block position -> sbuf block position."""
    return ki // 2 + 8 * (ki % 2)


def _build(niter=1, bias_q=False):
    nc = bacc.Bacc("TRN2", target_bir_lowering=False, debug=False, num_devices=N_CORES)

    xT_ext = nc.dram_tensor("xT", [(XO + S) * 128 * 8], FP8, kind="ExternalInput")
    wq_ext = nc.dram_tensor("wq", [128, 8, 128], FP8, kind="ExternalInput")
    wv_ext = nc.dram_tensor("wv", [128, 8, 128], FP8, kind="ExternalInput")
    bq_ext = (
        nc.dram_tensor("bq", [128, 1], FP32, kind="ExternalInput") if bias_q else None
    )
    mask_ext = nc.dram_tensor("masks", [128, 2, 128], BF16, kind="ExternalInput")
    out_ext = nc.dram_tensor("out", [QROWS, F], BF16, kind="ExternalOutput")

    with tile.TileContext(nc) as tc:
        with (
            tc.tile_pool(name="persist", bufs=1) as P,
            tc.tile_pool(name="ps_proj", bufs=2, space="PSUM") as ps_proj,
            tc.tile_pool(name="ps_log", bufs=2, space="PSUM") as ps_log,
            tc.tile_pool(name="ps_tp", bufs=1, space="PSUM") as ps_tp,
            tc.tile_pool(name="ps_read", bufs=1, space="PSUM") as ps_read,
            tc.For_i(0, niter) if niter > 1 else contextlib.nullcontext(),
        ):
            # ---- persistent SBUF tiles ----
            xT_sb = P.tile([128, 8, XO + S], FP8)  # [d%128, d//128, wk|x col]
            wq_sb = P.tile([128, 8, 128], FP8)
            wv_sb = P.tile([128, 8, 128], FP8)
            bq_sb = P.tile([128, 1], FP32) if bias_q else None
            mask_sb = P.tile([128, 2, 128], BF16)
            ident = P.tile([128, 128], BF16)
            zero_sb = P.tile([128, 1], FP32)
            kT_sb = P.tile([128, S], BF16)  # [feat, sbuf col] (x16 scaled)
            qT_sb = P.tile([128, QROWS], BF16)  # [feat, local q] (clean)
            vT_sb = P.tile([128, S], BF16)  # [feat, sbuf col] (x16 scaled)
            vaug_sb = P.tile([128, NKT, 144], FP8)  # [s%128, sbuf blk, v|16]
            expT_sb = P.tile([128, NKT, QROWS], FP8)  # [s%128, sbuf blk, q]
            read_sb = P.tile([128, NQT, 128], BF16)
            recip_sb = P.tile([128, NQT, 1], FP32)

            # ---- input DMAs spread over the three trigger queues ----
            xt_srcs = []
            base = 0
            for off, w in DCHUNKS:
                n = 128 * 8 * w
                xt_srcs.append(
                    xT_ext[base:base + n].rearrange("(p t w) -> p t w", p=128, t=8)
                )
                base += n

            def xdma(eng, i):
                off, w = DCHUNKS[i]
                eng.dma_start(xT_sb[:, :, off:off + w], xt_srcs[i])

            # HWDGE triggers serialize (~630 ns each): the first chunk
            # carries wk + the first x columns, so the first matmul's
            # operands arrive on a single trigger; remaining chunks
            # alternate sync/scalar rings in consumption order.
            xdma(nc.sync, 0)
            xdma(nc.scalar, 1)
            xdma(nc.sync, 2)
            xdma(nc.scalar, 3)
            xdma(nc.sync, 4)
            xdma(nc.scalar, 5)
            # SWDGE (gpsimd) runs on a parallel descriptor path: the
            # later-needed weights and masks go there.
            nc.gpsimd.dma_start(wq_sb[:], wq_ext[:])
            nc.gpsimd.dma_start(mask_sb[:], mask_ext[:])
            nc.gpsimd.dma_start(wv_sb[:], wv_ext[:])
            if bias_q:
                nc.gpsimd.dma_start(bq_sb[:], bq_ext[:])
            make_identity(nc, ident[:])
            nc.vector.memset(zero_sb[:], -2.0)
            nc.vector.memset(vaug_sb[:, :, 128:129], WSCALE)

            # ---- projections (fp8 DoubleRow: 2 contraction k-tiles per
            # pass, 4 passes for D=1024), PSUM evac on DVE except Q (ACT
            # applies bias + 1/16 descale) ----
            def proj(w_of, sl, dst_ap, evac):
                pp = ps_proj.tile([128, sl.stop - sl.start], FP32, tag="proj")
                for u in range(4):
                    nc.tensor.matmul(
                        pp[:],
                        w_of(u),
                        xT_sb[:, 2 * u:2 * u + 2, XO + sl.start:XO + sl.stop],
                        start=(u == 0),
                        stop=(u == 3),
                        perf_mode=DR,
                    )
                evac(dst_ap, pp)

            wk_of = lambda u: xT_sb[:, 2 * u:2 * u + 2, 0:128]
            wq_of = lambda u: wq_sb[:, 2 * u:2 * u + 2, :]
            wv_of = lambda u: wv_sb[:, 2 * u:2 * u + 2, :]

            def evac_dve(dst, pp):
                nc.vector.tensor_copy(dst, pp[:])

            def evac_act(dst, pp):
                nc.scalar.copy(dst, pp[:])

            if bias_q:
                def evac_q(dst, pp):
                    nc.scalar.activation(
                        dst, pp[:], mybir.ActivationFunctionType.Identity,
                        bias=bq_sb[:], scale=1.0 / WSCALE,
                    )
            else:
                def evac_q(dst, pp):
                    nc.vector.tensor_scalar_mul(dst, pp[:], 1.0 / WSCALE)

            # own half, in chunk order: K then Q per chunk (Q unblocks the
            # whole logits pipeline), then V
            for i in range(N_EVEN_CHUNKS):
                off, w = CHUNKS[i]
                sl = slice(off, off + w)
                proj(wk_of, sl, kT_sb[:, sl], evac_dve)
                proj(wq_of, sl, qT_sb[:, sl], evac_q)
            for off, w in ((0, 512), (512, 512)):
                sl = slice(off, off + w)
                proj(wv_of, sl, vT_sb[:, sl], evac_act)
                for pb in range(off // 128, (off + w) // 128):
                    pt = ps_tp.tile([128, 128], BF16, tag="tp")
                    nc.tensor.transpose(
                        pt[:], vT_sb[:, pb * 128:(pb + 1) * 128], ident[:]
                    )
                    nc.vector.tensor_copy(vaug_sb[:, pb, 0:128], pt[:])
            # other half: K then V per 512-chunk
            for off in (1024, 1536):
                sl = slice(off, off + 512)
                proj(wk_of, sl, kT_sb[:, sl], evac_dve)
                proj(wv_of, sl, vT_sb[:, sl], evac_act)
                for pb in range(off // 128, (off + 512) // 128):
                    pt = ps_tp.tile([128, 128], BF16, tag="tp")
                    nc.tensor.transpose(
                        pt[:], vT_sb[:, pb * 128:(pb + 1) * 128], ident[:]
                    )
                    nc.vector.tensor_copy(vaug_sb[:, pb, 0:128], pt[:])

            # ---- logits^T, exp, 0/1-mask, with each query block's PV
            # group emitted right after its last dependency (ki = 2j+1) so
            # PV matmuls don't queue behind later logits on the in-order
            # PE engine.  Masking expT on DVE keeps the mask OFF the
            # logits->exp critical chain (unmasked exp is safe: post-scale
            # logits are O(5)).  masks hold 0/1 here. ----
            out_read = out_ext[:].rearrange("(g p) c -> p g c", p=128)
            out_dmas = []
            expT_x = expT_sb[:].rearrange("p (two g) q -> p two g q", two=2)
            vaug_x = vaug_sb[:].rearrange("p (two g) c -> p two g c", two=2)
            for ki in range(NKT):
                p = p_of(ki)
                qs = 128 * (ki // 2)
                kb = slice(p * 128, (p + 1) * 128)
                off = qs
                while off < QROWS:
                    w = min(1024, QROWS - off)
                    pl = ps_log.tile([128, 1024], FP32, tag="log")
                    for so in range(0, w, 512):
                        sw = min(512, w - so)
                        nc.tensor.matmul(
                            pl[:, so:so + sw],
                            kT_sb[:, kb], qT_sb[:, off + so:off + so + sw],
                            start=True, stop=True,
                        )
                    nc.scalar.activation(
                        expT_sb[:, p, off:off + w], pl[:, 0:w],
                        mybir.ActivationFunctionType.Exp, bias=zero_sb[:],
                        scale=SCALE / WSCALE,
                    )
                    off += w
                nc.vector.tensor_mul(
                    expT_sb[:, p, qs:qs + 128],
                    expT_sb[:, p, qs:qs + 128],
                    mask_sb[:, ki % 2, :],
                )

                if ki % 2 == 0:
                    continue
            for ki in range(1, NKT, 2):
                # ---- PV + denominators + normalize for query block j ----
                j = ki // 2
                prt = ps_read.tile([128, 129], FP32, tag="read")
                pr = prt[:]
                # sbuf blocks [0..j] and [8..8+j]; DoubleRow-pair block m
                # with block m+8 (fp8: 2 key tiles per pass, no leftovers)
                jb = slice(j * 128, (j + 1) * 128)
                for m in range(j + 1):
                    nc.tensor.matmul(
                        pr,
                        expT_x[:, :, m, jb],
                        vaug_x[:, :, m, 0:129],
                        start=(m == 0),
                        stop=(m == j),
                        perf_mode=DR,
                    )
                nc.vector.reciprocal(recip_sb[:, j, :], pr[:, 128:129])
                nc.vector.tensor_scalar_mul(
                    read_sb[:, j, :], pr[:, 0:128], recip_sb[:, j, :]
                )
                # batched output writes: [0:4] on the parallel SWDGE path
                # mid-kernel, then [4:7] and the critical [7] on HWDGE
                if j == 3:
                    out_dmas.append(
                        nc.gpsimd.dma_start(
                            out=out_read[:, 0:4, :], in_=read_sb[:, 0:4, :]
                        )
                    )
                elif j == 7:
                    out_dmas.append(
                        nc.sync.dma_start(
                            out=out_read[:, 4:8, :], in_=read_sb[:, 4:8, :]
                        )
                    )

            if niter > 1:
                # timing mode: fence the iteration on every output DMA so the
                # per-iteration span matches a single-shot NEFF exec span
                fence = nc.vector.memset(zero_sb[:], -2.0)
                for d in out_dmas:
                    tile.add_dep_helper(fence.ins, d.ins, sync=True, reason="fence")

    nc.compile()
    return nc


def _get_compiled(niter=1, bias_q=False):
    key = f"nc{niter}b{int(bias_q)}"
    if key not in _compiled:
        _compiled[key] = _build(niter, bias_q=bias_q)
    return _compiled[key]


def _make_in_maps(inputs, Wq, bq, Wk, bk, Wv, bv):
    x = np.asarray(inputs, dtype=np.float32)
    assert x.shape == (B, S, D)

    def prep_w(w):
        w = (np.asarray(w, dtype=np.float32) * WSCALE).astype(FP8_NP)
        return np.ascontiguousarray(w.reshape(8, 128, 128).transpose(1, 0, 2))

    wq_np, wk_np, wv_np = prep_w(Wq), prep_w(Wk), prep_w(Wv)
    bias_q = bool(np.any(np.asarray(bq)))
    bq_np = np.asarray(bq, np.float32).reshape(128, 1)

    # 0/1 masks[k, slot, q]: slot 0 = diagonal block (triangular), slot 1 =
    # the extra block (all-0 for h=0, all-1 for h=1); multiplied into expT
    kk = np.arange(128)[:, None]
    qq = np.arange(128)[None, :]
    tri = (qq >= kk).astype(np.float32)
    m_h = []
    for h in range(2):
        other = np.full((128, 128), float(h), np.float32)
        m = np.stack([tri, other], axis=1)  # [k, slot, q]
        m_h.append(np.ascontiguousarray(m.astype(BF16_NP)))

    in_maps = []
    for c in range(N_CORES):
        b, h = divmod(c, 2)
        xb = x[b]  # [S, D]
        # sbuf block order: own parity blocks (2j+h) first, then the rest
        order = np.concatenate([np.arange(h, NKT, 2), np.arange(1 - h, NKT, 2)])
        xb_local = xb.reshape(NKT, 128, D)[order].reshape(S, D)
        xT_full = xb_local.T.astype(FP8_NP)  # [D, S] = [(t p), s]
        xT_tps = xT_full.reshape(8, 128, S).transpose(1, 0, 2)  # [p, t, s]
        xT_mega = np.concatenate([wk_np, xT_tps], axis=2)  # [p, t, wk|s]
        xT = np.concatenate(
            [xT_mega[:, :, off:off + w].reshape(-1) for off, w in DCHUNKS]
        )  # chunk-major flat, each chunk contiguous [p, t, w]
        m = {
            "xT": xT,
            "wq": wq_np,
            "wv": wv_np,
            "masks": m_h[h],
        }
        if bias_q:
            m["bq"] = bq_np
        in_maps.append(m)
    return in_maps, bias_q


def _gather(results, x, bv):
    """Assemble full output: exact host passthrough + device read part."""
    out = np.empty((B, S, D + F), dtype=np.float32)
    out[:, :, :D] = x
    bv = np.asarray(bv, np.float32)
    for c in range(N_CORES):
        b, h = divmod(c, 2)
        oc = np.asarray(results[c]["out"], dtype=np.float32).reshape(NQT, 128, F)
        for j in range(NQT):
            g = 2 * j + h
            out[b, g * 128:(g + 1) * 128, D:] = oc[j] + bv
    return out


def run(inputs, Wq, bq, Wk, bk, Wv, bv, trace=False):
    """Build (cached), run on 8 cores, gather. Returns (output, results)."""
    in_maps, bias_q = _make_in_maps(inputs, Wq, bq, Wk, bk, Wv, bv)
    nc = _get_compiled(bias_q=bias_q)
    x = np.asarray(inputs, dtype=np.float32)
    if trace:
        try:
            res = run_bass_kernel_spmd(nc, in_maps, list(range(N_CORES)), trace=True)
            return _gather(res.results, x, bv), res
        except Exception as e:  # profiling hook unavailable etc.
            print(f"trace run failed ({e!r}); falling back to untraced run")
    res = run_bass_kernel_spmd(nc, in_maps, list(range(N_CORES)))
    return _gather(res.results, x, bv), res


def kernel(inputs, Wq, bq, Wk, bk, Wv, bv):
    out, _ = run(inputs, Wq, bq, Wk, bk, Wv, bv, trace=False)
    return out
